# revision 18
# baseline (speedup 1.0000x reference)
"""AttentionLSTM Trainium2 kernel: data-parallel over batch on 8 NeuronCores.

Reference semantics (per batch element n):
  A_flat = A.reshape(N, H, 16); h0 = c0 = mean_p(A_flat)
  xWx = x @ Wx
  per step t:
    scores[p] = (h . A_flat[:, p]) / sqrt(H)
    w = softmax(scores); attn = A_flat @ w
    a = xWx_t + h @ Wh + attn @ Wattn + b
    i,f,o,g = sig/sig/sig/tanh of quarters; c = f*c + i*g; h = o*tanh(c)
  out[:, t, :] = h

Shapes: N=512, T=64, D=512, H=512 (4H=2048). 8 cores, 64 batch each.

Per core, the 64-element batch is split into TWO independent 32-element
STREAMS, phase-staggered so that while one stream's serial
scores->softmax->attention chain runs on Vector/Scalar, the other
stream's dense GEMM work keeps the PE array continuously busy (the HAM
clock gate re-throttles the PE to 1.2 GHz after idle windows, so
sustained PE activity is worth 2x in clock alone).

Every N=512 matmul is a 4-way column-tiled QUAD: the same [128,32]
stationary is loaded at array column offsets 0/32/64/96, and the four
concurrent streams carry the four E-chunks (i,f,g,o) of the fused
weight, all accumulating into the SAME psum bank at the same column
offsets (complementary partition ranges merge in the drain). Each
partition-range's accumulation group is complete on its own (has_written
clears are per-written-element, verified on HW), so no combine step is
needed. Gates are read from their partition quarter and written to
partitions 0-31 by ScalarE's cross-partition ACT (verified on HW).

  - scores: X[m,(p,n)] quad -> cross-partition mask-mul -> grouped reduce.
  - softmax kept on the sigmoid ACT table: e^x = sig(x)/(1-sig(x)).
  - wS transpose via single DVE 32x32 stream-transpose (no PE/PSUM trip).
  - attn: block-diag matmul, stationary A_PT, moving wBD 8-col blocks.
  - state update (f*c, i*g, +) on GpSimd, freeing Vector for the chains.
"""

import math
import sys

sys.path.insert(0, "/opt/trn_rl_repo")

import numpy as np
import ml_dtypes

import concourse.bass as bass
import concourse.mybir as mybir
from concourse.tile import TileContext
from concourse.bass_utils import run_bass_kernel_spmd

N, T, D, H = 512, 64, 512, 512
E = 4 * H  # 2048
NCORES = 8
NL = N // NCORES  # 64 batch per core
B = 32  # batch per stream
P16 = 16  # attention positions
NB = 4  # batch blocks of 8 per stream
SCALE = 1.0 / math.sqrt(H)

F32 = mybir.dt.float32
MM_DT = mybir.dt.bfloat16

# E-chunk quarters [i|f|o|g] and their quad column positions
CH = {"i": 0, "f": 1, "o": 2, "g": 3}
POS = {"i": 0, "f": 32, "g": 64, "o": 96}  # psum row offset = col position


def build_nc(reps=1):
    nc = bass.Bass("TRN2", target_bir_lowering=False)

    # --- DRAM I/O ---
    xT_d = nc.declare_dram_parameter("xT", [T, D, NL], MM_DT, isOutput=False)
    AhT_d = nc.declare_dram_parameter("AhT", [H, 2, P16 * B], MM_DT, isOutput=False)
    APT_d = nc.declare_dram_parameter("APT", [128, 2, NB, H], MM_DT, isOutput=False)
    W_d = nc.declare_dram_parameter("W", [3 * H, E], MM_DT, isOutput=False)
    b_d = nc.declare_dram_parameter("bias", [1, E], MM_DT, isOutput=False)
    h0_d = nc.declare_dram_parameter("h0", [NL, H], F32, isOutput=False)
    h0T_d = nc.declare_dram_parameter("h0T", [H, NL], MM_DT, isOutput=False)
    i32_d = nc.declare_dram_parameter("i32", [B, B], F32, isOutput=False)
    d16_d = nc.declare_dram_parameter("d16", [P16, 128], MM_DT, isOutput=False)
    mPN_d = nc.declare_dram_parameter("mPN", [128, 4 * B], F32, isOutput=False)
    mBD_d = nc.declare_dram_parameter("mBD", [128, B], MM_DT, isOutput=False)
    ones1_d = nc.declare_dram_parameter("ones1", [1, B], MM_DT, isOutput=False)
    out_d = nc.declare_dram_parameter("out", [NL, T, H], F32, isOutput=True)

    Sig = mybir.ActivationFunctionType.Sigmoid
    Tanh = mybir.ActivationFunctionType.Tanh

    with TileContext(nc) as tc:
        with (
            tc.tile_pool(name="wpool", bufs=1) as wpool,
            tc.tile_pool(name="state", bufs=1) as state,
            tc.tile_pool(name="xin", bufs=3) as xin,
            tc.tile_pool(name="work0", bufs=2) as work0,
            tc.tile_pool(name="work1", bufs=2) as work1,
            tc.tile_pool(name="psA", bufs=1, space="PSUM") as psA,
            tc.tile_pool(name="psB", bufs=1, space="PSUM") as psB,
        ):
            works = (work0, work1)
            # ---- persistent SBUF tensors (shared) ----
            W_sb = wpool.tile([128, 12, E], MM_DT, tag="W")
            nc.sync.dma_start(
                out=W_sb[:], in_=W_d.ap().rearrange("(k p) e -> p k e", p=128)
            )
            b_sb = wpool.tile([1, E], MM_DT, tag="bias")
            nc.sync.dma_start(out=b_sb[:], in_=b_d[:])
            AhT_sb = wpool.tile([128, 4, 2, P16 * B], MM_DT, tag="AhT")
            nc.sync.dma_start(
                out=AhT_sb[:], in_=AhT_d.ap().rearrange("(k p) s f -> p k s f", p=128)
            )
            APT_sb = wpool.tile([128, 2, NB, H], MM_DT, tag="APT")
            nc.sync.dma_start(out=APT_sb[:], in_=APT_d[:])
            i32_sb = wpool.tile([B, B], F32, tag="i32")
            nc.sync.dma_start(out=i32_sb[:], in_=i32_d[:])
            d16_sb = wpool.tile([P16, 128], MM_DT, tag="d16")
            nc.sync.dma_start(out=d16_sb[:], in_=d16_d[:])
            mPN_sb = wpool.tile([128, 4 * B], F32, tag="mPN")
            nc.sync.dma_start(out=mPN_sb[:], in_=mPN_d[:])
            mBD_sb = wpool.tile([128, B], MM_DT, tag="mBD")
            nc.sync.dma_start(out=mBD_sb[:], in_=mBD_d[:])
            ones1_sb = wpool.tile([1, B], MM_DT, tag="ones1")
            nc.sync.dma_start(out=ones1_sb[:], in_=ones1_d[:])

            # ---- per-stream state ----
            c_sb = [None, None]
            hT_sb = [None, None]
            for s in range(2):
                c_sb[s] = state.tile([B, H], F32, tag=f"c{s}", name=f"c{s}")
                nc.sync.dma_start(out=c_sb[s][:], in_=h0_d[s * B : (s + 1) * B, :])
                hT_sb[s] = state.tile([128, 4, B], MM_DT, tag=f"hT{s}", name=f"hT{s}")
                nc.sync.dma_start(
                    out=hT_sb[s][:],
                    in_=h0T_d[:, s * B : (s + 1) * B].rearrange(
                        "(k p) n -> p k n", p=128
                    ),
                )

            # ---- PSUM: per-stream gemm bank, X bank, misc bank ----
            # ms layout (fp32 words per partition): rep [0:32), hNT j at
            # [32+32j : 64+32j), at j at [160+32j : 192+32j). 288 words < 512.
            gm_ps = [psA.tile([128, 512], F32, tag=f"gm{s}", name=f"gm{s}") for s in range(2)]
            X_ps = [psA.tile([128, 512], F32, tag=f"X{s}", name=f"Xps{s}") for s in range(2)]
            ms_ps = [psB.tile([128, 512], F32, tag=f"ms{s}", name=f"ms{s}") for s in range(2)]

            def gemm_quad(s, k, stat, start, stop):
                """One K-tile for all four chunks, 4-way column-tiled into
                stream s's gemm bank (same 512-col window -> drains merge)."""
                for cn, cpos in POS.items():
                    cs = slice(CH[cn] * 512, (CH[cn] + 1) * 512)
                    nc.tensor.matmul(
                        gm_ps[s][cpos : cpos + B, :], stat, W_sb[:, k, cs],
                        start=start, stop=stop, tile_position=(0, cpos),
                    )

            def bias_quad(s):
                for cn, cpos in POS.items():
                    cs = slice(CH[cn] * 512, (CH[cn] + 1) * 512)
                    nc.tensor.matmul(
                        gm_ps[s][cpos : cpos + B, :], ones1_sb[:], b_sb[:, cs],
                        start=True, stop=False, tile_position=(0, cpos),
                    )

            def scores_quad(s, j):
                # X[m, (p, n)] quarters (128 cols each) at row quarters
                for q in range(4):
                    nc.tensor.matmul(
                        X_ps[s][q * B : (q + 1) * B, 0:128],
                        hT_sb[s][:, j],
                        AhT_sb[:, j, s, q * 128 : (q + 1) * 128],
                        start=(j == 0), stop=(j == 3),
                        tile_position=(0, q * B),
                    )

            _lp = tc.For_i(0, reps, 1) if reps > 1 else None
            if _lp is not None:
                _lp.__enter__()

            cur_x = xin.tile([128, 4, NL], MM_DT, tag="xT")
            nc.sync.dma_start(
                out=cur_x[:], in_=xT_d[0].rearrange("(k p) n -> p k n", p=128)
            )

            for t in range(T):
                nxt_x = None
                if t < T - 1:
                    nxt_x = xin.tile([128, 4, NL], MM_DT, tag="xT")
                    nc.sync.dma_start(
                        out=nxt_x[:],
                        in_=xT_d[t + 1].rearrange("(k p) n -> p k n", p=128),
                    )
                for s in range(2):
                    work = works[s]
                    xsl = slice(s * B, (s + 1) * B)

                    # ---- scores ----
                    for j in range(4):
                        scores_quad(s, j)

                    # ---- GEMM head: bias + xT/hT K-tiles (no attn dep) ----
                    bias_quad(s)
                    for k in range(4):
                        gemm_quad(s, k, cur_x[:, k, xsl], start=False, stop=False)
                    for k in range(4):
                        gemm_quad(s, 4 + k, hT_sb[s][:, k], start=False, stop=False)

                    # ---- diag extract: cross-partition mask-muls + reduce ----
                    Xm = work.tile([B, P16 * B], F32, tag="Xm")
                    for q in range(4):
                        nc.vector.tensor_mul(
                            Xm[:, q * 128 : (q + 1) * 128],
                            X_ps[s][q * B : (q + 1) * B, 0:128],
                            mPN_sb[q * B : (q + 1) * B, :],
                        )
                    scS = work.tile([B, P16], F32, tag="scS")
                    nc.vector.reduce_sum(
                        scS[:],
                        Xm[:].rearrange("q (p n) -> q p n", p=P16),
                        axis=mybir.AxisListType.X,
                    )
                    # ---- exp via sigmoid table ----
                    sg = work.tile([B, P16], F32, tag="sg")
                    nc.scalar.activation(sg[:], scS[:], Sig, scale=SCALE)
                    om = work.tile([B, P16], F32, tag="om")
                    nc.vector.tensor_scalar(
                        om[:], sg[:], -1.0, 1.0,
                        op0=mybir.AluOpType.mult, op1=mybir.AluOpType.add,
                    )
                    omr = work.tile([B, P16], F32, tag="omr")
                    nc.vector.reciprocal(omr[:], om[:])
                    expS = work.tile([B, P16], F32, tag="expS")
                    nc.vector.tensor_mul(expS[:], sg[:], omr[:])
                    den = work.tile([B, 1], F32, tag="den")
                    nc.vector.reduce_sum(den[:], expS[:], axis=mybir.AxisListType.X)
                    rd = work.tile([B, 1], F32, tag="rd")
                    nc.vector.reciprocal(rd[:], den[:])
                    wS = work.tile([B, B], MM_DT, tag="wS")
                    nc.vector.tensor_scalar_mul(wS[:, 0:P16], expS[:], rd[:])

                    # ---- wST via one DVE 32x32 block transpose ----
                    wST = work.tile([B, B], MM_DT, tag="wST")
                    nc.vector.transpose(wST[:], wS[:])
                    # ---- replicate p-rows x8, mask -> wBD ----
                    rep_ps = ms_ps[s][:, 0:B]
                    nc.tensor.matmul(
                        rep_ps, d16_sb[:], wST[0:P16, :], start=True, stop=True
                    )
                    wBD = work.tile([128, B], MM_DT, tag="wBD")
                    nc.vector.tensor_mul(wBD[:], rep_ps, mBD_sb[:])

                    # ---- attnT: block-diag matmuls, evac per j-tile ----
                    attnT = work.tile([128, 4, B], MM_DT, tag="attnT")
                    for j in range(4):
                        at_j = ms_ps[s][:, 5 * B + j * B : 5 * B + (j + 1) * B]
                        for bb in range(NB):
                            nc.tensor.matmul(
                                at_j[:, bb * 8 : (bb + 1) * 8],
                                APT_sb[:, s, bb, j * 128 : (j + 1) * 128],
                                wBD[:, bb * 8 : (bb + 1) * 8],
                                start=True,
                                stop=True,
                            )
                        nc.vector.tensor_copy(attnT[:, j], at_j)

                    # ---- GEMM attn K-tiles (close the bank's groups) ----
                    for k in range(8, 12):
                        gemm_quad(s, k, attnT[:, k - 8],
                                  start=False, stop=(k == 11))

                    # ---- gates + state ----
                    ig = work.tile([B, H], F32, tag="ig")
                    fg = work.tile([B, H], F32, tag="fg")
                    gg = work.tile([B, H], F32, tag="gg")
                    og = work.tile([B, H], F32, tag="og")
                    nc.scalar.activation(
                        ig[:], gm_ps[s][POS["i"] : POS["i"] + B, :], Sig
                    )
                    nc.scalar.activation(
                        fg[:], gm_ps[s][POS["f"] : POS["f"] + B, :], Sig
                    )
                    fcp = work.tile([B, H], F32, tag="fcp")
                    nc.gpsimd.tensor_mul(fcp[:], fg[:], c_sb[s][:])
                    nc.scalar.activation(
                        gg[:], gm_ps[s][POS["g"] : POS["g"] + B, :], Tanh
                    )
                    nc.scalar.activation(
                        og[:], gm_ps[s][POS["o"] : POS["o"] + B, :], Sig
                    )
                    igp = work.tile([B, H], F32, tag="igp")
                    nc.gpsimd.tensor_mul(igp[:], ig[:], gg[:])
                    nc.gpsimd.tensor_add(c_sb[s][:], fcp[:], igp[:])
                    tc_sb = work.tile([B, H], F32, tag="tc")
                    hN = work.tile([B, H], F32, tag="hN")
                    for j in range(4):
                        js = slice(j * 128, (j + 1) * 128)
                        nc.scalar.activation(tc_sb[:, js], c_sb[s][:, js], Tanh)
                        nc.vector.tensor_mul(hN[:, js], og[:, js], tc_sb[:, js])
                        if t < T - 1:
                            tp_ps = ms_ps[s][:, B + j * B : B + (j + 1) * B]
                            nc.tensor.transpose(tp_ps, hN[:, js], i32_sb[:])
                            nc.scalar.copy(hT_sb[s][:, j], tp_ps)

                    # ---- DMA out ----
                    nc.sync.dma_start(out=out_d[xsl, t, :], in_=hN[:])

                if nxt_x is not None:
                    cur_x = nxt_x

            if _lp is not None:
                _lp.__exit__(None, None, None)

    _split_matmul_waits(nc)
    return nc


def _split_matmul_waits(nc):
    """Several TPB instruction encodings accept only one sync-wait command;
    hoist excess waits onto an inserted same-engine drain."""
    cnt = 0
    for f in nc.m.functions:
        for blk in f.blocks:
            new_insts = []
            for ins in blk.instructions:
                if (
                    ins.sync_info is not None
                    and ins.sync_info.on_wait
                    and len(ins.sync_info.on_wait) > 1
                ):
                    waits = list(ins.sync_info.on_wait)
                    for w in waits[:-1]:
                        cnt += 1
                        d = mybir.InstDrain(
                            name=f"I-mmw{cnt}", ins=[], outs=[],
                            engine=ins.engine,
                        )
                        d.sync_info = mybir.SyncInfo(on_wait=[w], on_update=[])
                        new_insts.append(d)
                    ins.sync_info = mybir.SyncInfo(
                        on_wait=[waits[-1]], on_update=list(ins.sync_info.on_update or [])
                    )
                new_insts.append(ins)
            blk.instructions = new_insts


def _prep_core_inputs(x_i, A_i, Wx, Wh, Wattn, b):
    """Host-side layout prep for one core's shard (x_i: (64,T,D), A_i: (64,H,4,4))."""
    nl = x_i.shape[0]
    A_flat = A_i.reshape(nl, H, P16)
    h0 = A_flat.mean(axis=2).astype(np.float32)  # (64, H)

    xT = np.ascontiguousarray(x_i.transpose(1, 2, 0)).astype(np.float32)  # (T, D, 64)
    # AhT[h, s, p*32+n] = A_flat[32s+n, h, p]
    AhT = np.ascontiguousarray(
        A_flat.transpose(1, 2, 0).reshape(H, P16, 2, B).transpose(0, 2, 1, 3)
        .reshape(H, 2, P16 * B)
    ).astype(np.float32)
    # APT[p*8+r, s, b, h] = A_flat[32s + 8b + r, h, p]
    APT = np.ascontiguousarray(
        A_flat.reshape(2, NB, 8, H, P16).transpose(4, 2, 0, 1, 3)
        .reshape(128, 2, NB, H)
    ).astype(np.float32)
    W = np.concatenate([Wx, Wh, Wattn], axis=0).astype(np.float32)  # (1536, E)
    i32 = np.eye(B, dtype=np.float32)
    d16 = np.repeat(np.eye(P16, dtype=np.float32), 8, axis=1)  # (16, 128)
    # mPN[q*32+m, p_local*32+n] = (n == m)
    mPN = np.tile(np.tile(np.eye(B, dtype=np.float32), (1, 4)), (4, 1))  # (128, 128)
    mBD = np.tile(np.tile(np.eye(8, dtype=np.float32), (1, NB)), (P16, 1))  # (128,32)
    ones1 = np.ones((1, B), dtype=np.float32)
    bf16 = ml_dtypes.bfloat16
    return {
        "xT": xT.astype(bf16),
        "AhT": AhT.astype(bf16),
        "APT": APT.astype(bf16),
        "W": W.astype(bf16),
        "bias": b.reshape(1, E).astype(bf16),
        "h0": h0,
        "h0T": np.ascontiguousarray(h0.T).astype(bf16),
        "i32": i32,
        "d16": d16.astype(bf16),
        "mPN": mPN,
        "mBD": mBD.astype(bf16),
        "ones1": ones1.astype(bf16),
    }


_NC_CACHE = {}


def kernel(x, A, Wx, Wh, Wattn, b, _trace=False):
    x = np.asarray(x, dtype=np.float32)
    A = np.asarray(A, dtype=np.float32)
    Wx = np.asarray(Wx, dtype=np.float32)
    Wh = np.asarray(Wh, dtype=np.float32)
    Wattn = np.asarray(Wattn, dtype=np.float32)
    b = np.asarray(b, dtype=np.float32)

    if "nc" not in _NC_CACHE:
        _NC_CACHE["nc"] = build_nc()
    nc = _NC_CACHE["nc"]

    in_maps = []
    for i in range(NCORES):
        sl = slice(i * NL, (i + 1) * NL)
        in_maps.append(_prep_core_inputs(x[sl], A[sl], Wx, Wh, Wattn, b))

    res = run_bass_kernel_spmd(
        nc, in_maps, core_ids=list(range(NCORES)), trace=_trace
    )
    outs = [res.results[i]["out"] for i in range(NCORES)]
    full = np.concatenate(outs, axis=0)  # (N, T, H)
    if _trace:
        kernel.last_exec_time_ns = res.exec_time_ns
        kernel.last_profile = res.profile_json
    return full


kernel.last_exec_time_ns = None
kernel.last_profile = None


# revision 20
# speedup vs baseline: 1.1079x; 1.1079x over previous
"""AttentionLSTM Trainium2 kernel: data-parallel over batch on 8 NeuronCores.

Reference semantics (per batch element n):
  A_flat = A.reshape(N, H, 16); h0 = c0 = mean_p(A_flat)
  xWx = x @ Wx
  per step t:
    scores[p] = (h . A_flat[:, p]) / sqrt(H)
    w = softmax(scores); attn = A_flat @ w
    a = xWx_t + h @ Wh + attn @ Wattn + b
    i,f,o,g = sig/sig/sig/tanh of quarters; c = f*c + i*g; h = o*tanh(c)
  out[:, t, :] = h

Shapes: N=512, T=64, D=512, H=512 (4H=2048). 8 cores, 64 batch each.

Per core, the 64-element batch is split into TWO independent 32-element
STREAMS, phase-staggered so that while one stream's serial
scores->softmax->attention chain runs on Vector/Scalar, the other
stream's dense GEMM work keeps the PE array continuously busy (the HAM
clock gate re-throttles the PE to 1.2 GHz after idle windows, so
sustained PE activity is worth 2x in clock alone).

Every N=512 matmul is a 4-way column-tiled QUAD: the same [128,32]
stationary is loaded at array column offsets 0/32/64/96, and the four
concurrent streams carry the four E-chunks (i,f,g,o) of the fused
weight, all accumulating into the SAME psum bank at the same column
offsets (complementary partition ranges merge in the drain). Each
partition-range's accumulation group is complete on its own (has_written
clears are per-written-element, verified on HW), so no combine step is
needed. Gates are read from their partition quarter and written to
partitions 0-31 by ScalarE's cross-partition ACT (verified on HW).

  - scores: X[m,(p,n)] quad -> cross-partition mask-mul -> grouped reduce.
  - softmax kept on the sigmoid ACT table: e^x = sig(x)/(1-sig(x)).
  - wS transpose via single DVE 32x32 stream-transpose (no PE/PSUM trip).
  - attn: block-diag matmul, stationary A_PT, moving wBD 8-col blocks.
  - state update (f*c, i*g, +) on GpSimd, freeing Vector for the chains.
"""

import math
import sys

sys.path.insert(0, "/opt/trn_rl_repo")

import numpy as np
import ml_dtypes

import concourse.bass as bass
import concourse.mybir as mybir
from concourse.tile import TileContext
from concourse.bass_utils import run_bass_kernel_spmd

N, T, D, H = 512, 64, 512, 512
E = 4 * H  # 2048
NCORES = 8
NL = N // NCORES  # 64 batch per core
B = 32  # batch per stream
P16 = 16  # attention positions
NB = 4  # batch blocks of 8 per stream
SCALE = 1.0 / math.sqrt(H)

F32 = mybir.dt.float32
MM_DT = mybir.dt.bfloat16

# E-chunk quarters [i|f|o|g] and their quad column positions
CH = {"i": 0, "f": 1, "o": 2, "g": 3}
POS = {"i": 0, "f": 32, "g": 64, "o": 96}  # psum row offset = col position


def build_nc(reps=1):
    nc = bass.Bass("TRN2", target_bir_lowering=False)

    # --- DRAM I/O ---
    xT_d = nc.declare_dram_parameter("xT", [T, D, NL], MM_DT, isOutput=False)
    AhT_d = nc.declare_dram_parameter("AhT", [H, 2, P16 * B], MM_DT, isOutput=False)
    APT_d = nc.declare_dram_parameter("APT", [128, 2, NB, H], MM_DT, isOutput=False)
    W_d = nc.declare_dram_parameter("W", [3 * H, E], MM_DT, isOutput=False)
    b_d = nc.declare_dram_parameter("bias", [1, E], MM_DT, isOutput=False)
    h0_d = nc.declare_dram_parameter("h0", [NL, H], F32, isOutput=False)
    h0T_d = nc.declare_dram_parameter("h0T", [H, NL], MM_DT, isOutput=False)
    i32_d = nc.declare_dram_parameter("i32", [B, B], F32, isOutput=False)
    d16_d = nc.declare_dram_parameter("d16", [P16, 128], MM_DT, isOutput=False)
    mPN_d = nc.declare_dram_parameter("mPN", [128, 4 * B], F32, isOutput=False)
    mBD_d = nc.declare_dram_parameter("mBD", [128, B], MM_DT, isOutput=False)
    ones1_d = nc.declare_dram_parameter("ones1", [1, B], MM_DT, isOutput=False)
    out_d = nc.declare_dram_parameter("out", [NL, T, H], F32, isOutput=True)

    Sig = mybir.ActivationFunctionType.Sigmoid
    Tanh = mybir.ActivationFunctionType.Tanh

    with TileContext(nc) as tc:
        with (
            tc.tile_pool(name="wpool", bufs=1) as wpool,
            tc.tile_pool(name="state", bufs=1) as state,
            tc.tile_pool(name="xin", bufs=3) as xin,
            tc.tile_pool(name="work0", bufs=2) as work0,
            tc.tile_pool(name="work1", bufs=2) as work1,
            tc.tile_pool(name="psA", bufs=1, space="PSUM") as psA,
            tc.tile_pool(name="psB", bufs=1, space="PSUM") as psB,
        ):
            works = (work0, work1)
            # ---- persistent SBUF tensors (shared) ----
            W_sb = wpool.tile([128, 12, E], MM_DT, tag="W")
            nc.sync.dma_start(
                out=W_sb[:], in_=W_d.ap().rearrange("(k p) e -> p k e", p=128)
            )
            b_sb = wpool.tile([1, E], MM_DT, tag="bias")
            nc.sync.dma_start(out=b_sb[:], in_=b_d[:])
            AhT_sb = wpool.tile([128, 4, 2, P16 * B], MM_DT, tag="AhT")
            nc.sync.dma_start(
                out=AhT_sb[:], in_=AhT_d.ap().rearrange("(k p) s f -> p k s f", p=128)
            )
            APT_sb = wpool.tile([128, 2, NB, H], MM_DT, tag="APT")
            nc.sync.dma_start(out=APT_sb[:], in_=APT_d[:])
            i32_sb = wpool.tile([B, B], F32, tag="i32")
            nc.sync.dma_start(out=i32_sb[:], in_=i32_d[:])
            d16_sb = wpool.tile([P16, 128], MM_DT, tag="d16")
            nc.sync.dma_start(out=d16_sb[:], in_=d16_d[:])
            mPN_sb = wpool.tile([128, 4 * B], F32, tag="mPN")
            nc.sync.dma_start(out=mPN_sb[:], in_=mPN_d[:])
            mBD_sb = wpool.tile([128, B], MM_DT, tag="mBD")
            nc.sync.dma_start(out=mBD_sb[:], in_=mBD_d[:])
            ones1_sb = wpool.tile([1, B], MM_DT, tag="ones1")
            nc.sync.dma_start(out=ones1_sb[:], in_=ones1_d[:])

            # ---- per-stream state ----
            c_sb = [None, None]
            hT_sb = [None, None]
            for s in range(2):
                c_sb[s] = state.tile([B, H], F32, tag=f"c{s}", name=f"c{s}")
                nc.sync.dma_start(out=c_sb[s][:], in_=h0_d[s * B : (s + 1) * B, :])
                hT_sb[s] = state.tile([128, 4, B], MM_DT, tag=f"hT{s}", name=f"hT{s}")
                nc.sync.dma_start(
                    out=hT_sb[s][:],
                    in_=h0T_d[:, s * B : (s + 1) * B].rearrange(
                        "(k p) n -> p k n", p=128
                    ),
                )

            # ---- PSUM: per-stream gemm bank, X bank, misc bank ----
            # ms layout (fp32 words per partition): rep [0:32), hNT j at
            # [32+32j : 64+32j), at j at [160+32j : 192+32j). 288 words < 512.
            gm_ps = [psA.tile([128, 512], F32, tag=f"gm{s}", name=f"gm{s}") for s in range(2)]
            X_ps = [psA.tile([128, 512], F32, tag=f"X{s}", name=f"Xps{s}") for s in range(2)]
            ms_ps = [psB.tile([128, 512], F32, tag=f"ms{s}", name=f"ms{s}") for s in range(2)]

            def gemm_quad(s, k, stat, start, stop):
                """One K-tile for all four chunks, 4-way column-tiled into
                stream s's gemm bank (same 512-col window -> drains merge)."""
                for cn, cpos in POS.items():
                    cs = slice(CH[cn] * 512, (CH[cn] + 1) * 512)
                    nc.tensor.matmul(
                        gm_ps[s][cpos : cpos + B, :], stat, W_sb[:, k, cs],
                        start=start, stop=stop, tile_position=(0, cpos),
                    )

            def bias_quad(s):
                for cn, cpos in POS.items():
                    cs = slice(CH[cn] * 512, (CH[cn] + 1) * 512)
                    nc.tensor.matmul(
                        gm_ps[s][cpos : cpos + B, :], ones1_sb[:], b_sb[:, cs],
                        start=True, stop=False, tile_position=(0, cpos),
                    )

            def scores_quad(s, j):
                # X[m, (p, n)] quarters (128 cols each) at row quarters
                for q in range(4):
                    nc.tensor.matmul(
                        X_ps[s][q * B : (q + 1) * B, 0:128],
                        hT_sb[s][:, j],
                        AhT_sb[:, j, s, q * 128 : (q + 1) * 128],
                        start=(j == 0), stop=(j == 3),
                        tile_position=(0, q * B),
                    )

            def phase_a(ss, t, xt):
                """Chain-independent front half of stream ss's step t: scores,
                bias + xT/hT GEMM K-tiles, then the softmax chain on DVE/ACT.
                The PE work here covers the OTHER stream's chain."""
                work = works[ss]
                for j in range(4):
                    scores_quad(ss, j)
                bias_quad(ss)
                for k in range(4):
                    gemm_quad(ss, k, xt[:, k, ss * B : (ss + 1) * B],
                              start=False, stop=False)
                for k in range(4):
                    gemm_quad(ss, 4 + k, hT_sb[ss][:, k], start=False, stop=False)

                # ---- diag extract: cross-partition mask-muls + reduce ----
                Xm = work.tile([B, P16 * B], F32, tag="Xm")
                for q in range(4):
                    nc.vector.tensor_mul(
                        Xm[:, q * 128 : (q + 1) * 128],
                        X_ps[ss][q * B : (q + 1) * B, 0:128],
                        mPN_sb[q * B : (q + 1) * B, :],
                    )
                scS = work.tile([B, P16], F32, tag="scS")
                nc.vector.reduce_sum(
                    scS[:],
                    Xm[:].rearrange("q (p n) -> q p n", p=P16),
                    axis=mybir.AxisListType.X,
                )
                # ---- exp via sigmoid table ----
                sg = work.tile([B, P16], F32, tag="sg")
                nc.scalar.activation(sg[:], scS[:], Sig, scale=SCALE)
                om = work.tile([B, P16], F32, tag="om")
                nc.vector.tensor_scalar(
                    om[:], sg[:], -1.0, 1.0,
                    op0=mybir.AluOpType.mult, op1=mybir.AluOpType.add,
                )
                omr = work.tile([B, P16], F32, tag="omr")
                nc.vector.reciprocal(omr[:], om[:])
                expS = work.tile([B, P16], F32, tag="expS")
                nc.vector.tensor_mul(expS[:], sg[:], omr[:])
                den = work.tile([B, 1], F32, tag="den")
                nc.vector.reduce_sum(den[:], expS[:], axis=mybir.AxisListType.X)
                rd = work.tile([B, 1], F32, tag="rd")
                nc.vector.reciprocal(rd[:], den[:])
                wS = work.tile([B, B], MM_DT, tag="wS")
                nc.vector.tensor_scalar_mul(wS[:, 0:P16], expS[:], rd[:])
                wST = work.tile([B, B], MM_DT, tag="wST")
                nc.vector.transpose(wST[:], wS[:])
                return wST

            def phase_b(ss, t, wST):
                """Chain-dependent back half: replicate weights, block-diag
                attention, attn GEMM K-tiles, gates, state update, h
                transposes. Covered by the OTHER stream's phase A."""
                work = works[ss]
                rep_ps = ms_ps[ss][:, 0:B]
                nc.tensor.matmul(
                    rep_ps, d16_sb[:], wST[0:P16, :], start=True, stop=True
                )
                wBD = work.tile([128, B], MM_DT, tag="wBD")
                nc.vector.tensor_mul(wBD[:], rep_ps, mBD_sb[:])

                attnT = work.tile([128, 4, B], MM_DT, tag="attnT")
                for j in range(4):
                    at_j = ms_ps[ss][:, 5 * B + j * B : 5 * B + (j + 1) * B]
                    for bb in range(NB):
                        nc.tensor.matmul(
                            at_j[:, bb * 8 : (bb + 1) * 8],
                            APT_sb[:, ss, bb, j * 128 : (j + 1) * 128],
                            wBD[:, bb * 8 : (bb + 1) * 8],
                            start=True,
                            stop=True,
                        )
                    nc.vector.tensor_copy(attnT[:, j], at_j)

                for k in range(8, 12):
                    gemm_quad(ss, k, attnT[:, k - 8], start=False, stop=(k == 11))

                ig = work.tile([B, H], F32, tag="ig")
                fg = work.tile([B, H], F32, tag="fg")
                gg = work.tile([B, H], F32, tag="gg")
                og = work.tile([B, H], F32, tag="og")
                nc.scalar.activation(
                    ig[:], gm_ps[ss][POS["i"] : POS["i"] + B, :], Sig
                )
                nc.scalar.activation(
                    fg[:], gm_ps[ss][POS["f"] : POS["f"] + B, :], Sig
                )
                fcp = work.tile([B, H], F32, tag="fcp")
                nc.gpsimd.tensor_mul(fcp[:], fg[:], c_sb[ss][:])
                nc.scalar.activation(
                    gg[:], gm_ps[ss][POS["g"] : POS["g"] + B, :], Tanh
                )
                nc.scalar.activation(
                    og[:], gm_ps[ss][POS["o"] : POS["o"] + B, :], Sig
                )
                igp = work.tile([B, H], F32, tag="igp")
                nc.gpsimd.tensor_mul(igp[:], ig[:], gg[:])
                nc.gpsimd.tensor_add(c_sb[ss][:], fcp[:], igp[:])
                tc_sb = work.tile([B, H], F32, tag="tc")
                hN = work.tile([B, H], F32, tag="hN")
                for j in range(4):
                    js = slice(j * 128, (j + 1) * 128)
                    nc.scalar.activation(tc_sb[:, js], c_sb[ss][:, js], Tanh)
                    nc.vector.tensor_mul(hN[:, js], og[:, js], tc_sb[:, js])
                    if t < T - 1:
                        tp_ps = ms_ps[ss][:, B + j * B : B + (j + 1) * B]
                        nc.tensor.transpose(tp_ps, hN[:, js], i32_sb[:])
                        nc.scalar.copy(hT_sb[ss][:, j], tp_ps)
                nc.sync.dma_start(
                    out=out_d[ss * B : (ss + 1) * B, t, :], in_=hN[:]
                )

            _lp = tc.For_i(0, reps, 1) if reps > 1 else None
            if _lp is not None:
                _lp.__enter__()

            xts = {}
            xt0 = xin.tile([128, 4, NL], MM_DT, tag="xT")
            xts[0] = xt0
            nc.sync.dma_start(
                out=xt0[:], in_=xT_d[0].rearrange("(k p) n -> p k n", p=128)
            )

            # Half-step slot pipeline: slot u runs phase A of stream u%2 at
            # step u//2, then phase B of the OTHER stream lagging half a slot.
            # The static per-engine order this produces is what staggers the
            # two streams on hardware.
            wst = {}
            for u in range(2 * T + 1):
                sa, ta = u % 2, u // 2
                if ta < T:
                    if ta + 1 < T and (ta + 1) not in xts and sa == 0:
                        xtn = xin.tile([128, 4, NL], MM_DT, tag="xT")
                        xts[ta + 1] = xtn
                        nc.sync.dma_start(
                            out=xtn[:],
                            in_=xT_d[ta + 1].rearrange("(k p) n -> p k n", p=128),
                        )
                    wst[(sa, ta)] = phase_a(sa, ta, xts[ta])
                if u >= 1:
                    sb, tb = 1 - (u % 2), (u - 1) // 2
                    if tb < T:
                        phase_b(sb, tb, wst.pop((sb, tb)))

            if _lp is not None:
                _lp.__exit__(None, None, None)

    _split_matmul_waits(nc)
    return nc


def _split_matmul_waits(nc):
    """Several TPB instruction encodings accept only one sync-wait command;
    hoist excess waits onto an inserted same-engine drain."""
    cnt = 0
    for f in nc.m.functions:
        for blk in f.blocks:
            new_insts = []
            for ins in blk.instructions:
                if (
                    ins.sync_info is not None
                    and ins.sync_info.on_wait
                    and len(ins.sync_info.on_wait) > 1
                ):
                    waits = list(ins.sync_info.on_wait)
                    for w in waits[:-1]:
                        cnt += 1
                        d = mybir.InstDrain(
                            name=f"I-mmw{cnt}", ins=[], outs=[],
                            engine=ins.engine,
                        )
                        d.sync_info = mybir.SyncInfo(on_wait=[w], on_update=[])
                        new_insts.append(d)
                    ins.sync_info = mybir.SyncInfo(
                        on_wait=[waits[-1]], on_update=list(ins.sync_info.on_update or [])
                    )
                new_insts.append(ins)
            blk.instructions = new_insts


def _prep_core_inputs(x_i, A_i, Wx, Wh, Wattn, b):
    """Host-side layout prep for one core's shard (x_i: (64,T,D), A_i: (64,H,4,4))."""
    nl = x_i.shape[0]
    A_flat = A_i.reshape(nl, H, P16)
    h0 = A_flat.mean(axis=2).astype(np.float32)  # (64, H)

    xT = np.ascontiguousarray(x_i.transpose(1, 2, 0)).astype(np.float32)  # (T, D, 64)
    # AhT[h, s, p*32+n] = A_flat[32s+n, h, p]
    AhT = np.ascontiguousarray(
        A_flat.transpose(1, 2, 0).reshape(H, P16, 2, B).transpose(0, 2, 1, 3)
        .reshape(H, 2, P16 * B)
    ).astype(np.float32)
    # APT[p*8+r, s, b, h] = A_flat[32s + 8b + r, h, p]
    APT = np.ascontiguousarray(
        A_flat.reshape(2, NB, 8, H, P16).transpose(4, 2, 0, 1, 3)
        .reshape(128, 2, NB, H)
    ).astype(np.float32)
    W = np.concatenate([Wx, Wh, Wattn], axis=0).astype(np.float32)  # (1536, E)
    i32 = np.eye(B, dtype=np.float32)
    d16 = np.repeat(np.eye(P16, dtype=np.float32), 8, axis=1)  # (16, 128)
    # mPN[q*32+m, p_local*32+n] = (n == m)
    mPN = np.tile(np.tile(np.eye(B, dtype=np.float32), (1, 4)), (4, 1))  # (128, 128)
    mBD = np.tile(np.tile(np.eye(8, dtype=np.float32), (1, NB)), (P16, 1))  # (128,32)
    ones1 = np.ones((1, B), dtype=np.float32)
    bf16 = ml_dtypes.bfloat16
    return {
        "xT": xT.astype(bf16),
        "AhT": AhT.astype(bf16),
        "APT": APT.astype(bf16),
        "W": W.astype(bf16),
        "bias": b.reshape(1, E).astype(bf16),
        "h0": h0,
        "h0T": np.ascontiguousarray(h0.T).astype(bf16),
        "i32": i32,
        "d16": d16.astype(bf16),
        "mPN": mPN,
        "mBD": mBD.astype(bf16),
        "ones1": ones1.astype(bf16),
    }


_NC_CACHE = {}


def kernel(x, A, Wx, Wh, Wattn, b, _trace=False):
    x = np.asarray(x, dtype=np.float32)
    A = np.asarray(A, dtype=np.float32)
    Wx = np.asarray(Wx, dtype=np.float32)
    Wh = np.asarray(Wh, dtype=np.float32)
    Wattn = np.asarray(Wattn, dtype=np.float32)
    b = np.asarray(b, dtype=np.float32)

    if "nc" not in _NC_CACHE:
        _NC_CACHE["nc"] = build_nc()
    nc = _NC_CACHE["nc"]

    in_maps = []
    for i in range(NCORES):
        sl = slice(i * NL, (i + 1) * NL)
        in_maps.append(_prep_core_inputs(x[sl], A[sl], Wx, Wh, Wattn, b))

    res = run_bass_kernel_spmd(
        nc, in_maps, core_ids=list(range(NCORES)), trace=_trace
    )
    outs = [res.results[i]["out"] for i in range(NCORES)]
    full = np.concatenate(outs, axis=0)  # (N, T, H)
    if _trace:
        kernel.last_exec_time_ns = res.exec_time_ns
        kernel.last_profile = res.profile_json
    return full


kernel.last_exec_time_ns = None
kernel.last_profile = None
